# revision 1
# baseline (speedup 1.0000x reference)
"""CFConv (gnn_message_passing) Trainium2 kernel.

Strategy (self-contained, hardcoded for the nn_CFConv_5909874999436 shapes):
  h = segment_sum(node_weight[src] * MLP(rbf_out), dst, 100000)

- Host: sort edges by dst, partition the 100000 nodes into 8 contiguous
  ranges of 12500 (one per NeuronCore) -> fully independent cores, no
  collectives. Within a core, nodes are grouped into 98 windows of <=128
  nodes; each window's edges are padded to T tiles of 128 edge slots so all
  8 cores run one identical (SPMD) program.
- rbf features are pre-transposed on host to feature-major bf16 so the edge
  MLP runs without any on-chip transposes:
    mm1:  h1_T[64f, E] = W1(lhsT) @ x_T          (feature-major)
    act:  h1s = 2*log(1+exp(0.5*h1+0.5*b1))      (Exp then Ln, x2 folded in W2)
    mm2:  e[E, 64f] = h1s_T(lhsT) @ (2*W2)       (edge-major "for free")
    m = (e + b2) * gather(node_weight_bf16, src) (DVE)
    seg:  h_T[64f, 128n] += m(lhsT) @ onehot(dst_rel)   (PSUM accumulate)
- Windows are processed in pairs: pair member A lives on SBUF/PSUM
  partitions 0-63, member B on 64-127 (tile_position row/col groups), so
  DMA, ACT and PE all use the full 128-partition width.
"""

import os
import numpy as np
import ml_dtypes

bf16 = ml_dtypes.bfloat16

N_NODES = 100000
N_EDGES = 1600000
DIM = 64
CORES = 8
NPC = 12500          # nodes per core
WIN = 128            # nodes per window
NWIN = 98            # windows per core  (ceil(12500/128))
NPAIR = 49           # window pairs per core
BS = 6               # tiles per DVE batch

_NC_CACHE = {}
LAST_RESULTS = None  # set by kernel(); test.py reads exec time / trace from it


def _build(T, level=6, repeats=1, oh_engine="dve", badd="dve"):
    """level: cumulative stages for sim attribution.
    0: x-DMA only; 1:+mm1+act; 2:+gather; 3:+mm2; 4:+badd+mul; 5:+onehot;
    6: full (+seg+flush).  repeats>1 wraps the pair loop in a HW For_i."""
    import concourse.bacc as bacc
    import concourse.tile as tile
    from concourse import mybir
    from concourse.bass import IndirectOffsetOnAxis

    f32 = mybir.dt.float32
    bf = mybir.dt.bfloat16
    i32 = mybir.dt.int32
    i16 = mybir.dt.int16
    Act = mybir.ActivationFunctionType
    Alu = mybir.AluOpType

    S = NPAIR * T * 128       # xT columns per core
    WT = NWIN * T             # dst index columns per core
    GC = NPAIR * 2 * T * 64   # pre-gathered node-weight columns per core
    HC = NPAIR * 128          # output columns per core

    nc = bacc.Bacc("TRN2", target_bir_lowering=False, debug=False)
    xT = nc.dram_tensor("xT", [128, S], bf, kind="ExternalInput")
    nwp = nc.dram_tensor("nwp", [128, GC], bf, kind="ExternalInput")
    dwt = nc.dram_tensor("dstw", [128, WT], bf, kind="ExternalInput")
    W1 = nc.dram_tensor("W1r", [128, 64], bf, kind="ExternalInput")
    W2 = nc.dram_tensor("W2r", [128, 64], bf, kind="ExternalInput")
    B1 = nc.dram_tensor("b1h", [128, 1], f32, kind="ExternalInput")
    B2 = nc.dram_tensor("b2r", [128, BS * 64], f32, kind="ExternalInput")
    hout = nc.dram_tensor("h_out", [128, HC], f32, kind="ExternalOutput")

    # batches of tiles for the DVE stage
    batches = []
    t0 = 0
    while t0 < T:
        t1 = min(t0 + BS, T)
        batches.append((t0, t1))
        t0 = t1

    # mm1 half-pair width and column chunks (<=512 wide, PSUM-bank aligned)
    HALF = T * 64  # half of T*128, always a multiple of 64
    def _chunks(width):
        out, c = [], 0
        while c < width:
            out.append((c, min(c + 512, width)))
            c = min(c + 512, width)
        return out

    with tile.TileContext(nc) as tc:
        with (
            tc.tile_pool(name="const", bufs=1) as cpool,
            tc.tile_pool(name="xw", bufs=2) as xpool,
            tc.tile_pool(name="ex", bufs=2) as expool,
            tc.tile_pool(name="h1", bufs=2) as h1pool,
            tc.tile_pool(name="nwg", bufs=2) as nwpool,
            tc.tile_pool(name="eb", bufs=2) as ebpool,
            tc.tile_pool(name="mm", bufs=2) as mpool,
            tc.tile_pool(name="oh", bufs=2) as ohpool,
            tc.tile_pool(name="hsb", bufs=2) as hsbpool,
            tc.tile_pool(name="m1ps", bufs=1, space="PSUM") as m1pool,
            tc.tile_pool(name="eps", bufs=2, space="PSUM") as epool,
            tc.tile_pool(name="hps", bufs=1, space="PSUM") as hpool,
        ):
            w1sb = cpool.tile([128, 64], bf)
            nc.sync.dma_start(out=w1sb[:], in_=W1[:])
            w2sb = cpool.tile([128, 64], bf)
            nc.sync.dma_start(out=w2sb[:], in_=W2[:])
            b1sb = cpool.tile([128, 1], f32)
            nc.sync.dma_start(out=b1sb[:], in_=B1[:])
            b2sb = cpool.tile([128, BS * 64], f32)
            nc.sync.dma_start(out=b2sb[:], in_=B2[:])
            dwsb = cpool.tile([128, WT], bf)
            nc.sync.dma_start(out=dwsb[:], in_=dwt[:])
            onesc = cpool.tile([128, 128], bf)
            nc.vector.memset(onesc[:], 1.0)
            b2row = cpool.tile([1, BS * 64], bf)
            nc.vector.tensor_copy(out=b2row[:], in_=b2sb[:1, :])
            iota16 = cpool.tile([128, 128], i16)
            nc.gpsimd.iota(iota16[:], pattern=[[1, 128]], base=0, channel_multiplier=0)
            iota = cpool.tile([128, 128], bf)
            nc.vector.tensor_copy(out=iota[:], in_=iota16[:])

            def emit_pair(p):
                xw = xpool.tile([128, T * 128], bf)
                nc.sync.dma_start(out=xw[:], in_=xT[:, p * T * 128:(p + 1) * T * 128])

                if level < 1:
                    return
                ex = expool.tile([128, T * 128], f32)
                m1 = m1pool.tile([128, T * 128], f32, space="PSUM")
                for (c0, c1) in _chunks(T * 128):
                    nc.tensor.matmul(
                        out=m1[0:64, c0:c1], lhsT=w1sb[0:64, :], rhs=xw[0:64, c0:c1],
                        start=True, stop=True, tile_position=(0, 0),
                    )
                    nc.tensor.matmul(
                        out=m1[64:128, c0:c1], lhsT=w1sb[64:128, :], rhs=xw[64:128, c0:c1],
                        start=True, stop=True, tile_position=(64, 64),
                    )
                nc.scalar.activation(out=ex[:], in_=m1[:], func=Act.Exp,
                                     bias=b1sb[:], scale=0.5)
                h1 = h1pool.tile([128, T * 128], bf)
                nc.scalar.activation(out=h1[:], in_=ex[:], func=Act.Ln,
                                     bias=1.0, scale=1.0)

                if level < 2:
                    return
                nwpair = nwpool.tile([128, 2 * T * 64], bf)
                nc.sync.dma_start(
                    out=nwpair[:], in_=nwp[:, p * 2 * T * 64:(p + 1) * 2 * T * 64])
                hps = hpool.tile([128, 128], f32, space="PSUM")
                for (t0, t1) in batches:
                    bs = t1 - t0
                    if level < 3:
                        continue
                    epsh = []
                    for half in (0, 1):
                        r0 = 64 * half
                        eps = epool.tile([128, BS * 64], f32, space="PSUM")
                        epsh.append(eps)
                        if badd == "pe":
                            nc.tensor.matmul(
                                out=eps[:, :bs * 64], lhsT=onesc[r0:r0 + 1, :],
                                rhs=b2row[:, :bs * 64],
                                start=True, stop=False, skip_group_check=True,
                                tile_position=(r0, 0))
                    for j in range(bs):
                        t = t0 + j
                        for half in (0, 1):
                            r0 = 64 * half
                            nc.tensor.matmul(
                                out=epsh[half][:, j * 64:(j + 1) * 64],
                                lhsT=h1[r0:r0 + 64, t * 128:(t + 1) * 128],
                                rhs=w2sb[r0:r0 + 64, :],
                                start=(badd != "pe"), stop=True,
                                tile_position=(r0, 0),
                                skip_group_check=(badd == "pe"),
                            )
                    if level < 4:
                        continue
                    msbh = []
                    ohh = []
                    for half in (0, 1):
                        wc = (2 * p + half) * T
                        if badd == "pe":
                            mul_in = epsh[half]
                        else:
                            eb = ebpool.tile([128, BS * 64], bf)
                            nc.vector.tensor_tensor(
                                out=eb[:, :bs * 64], in0=epsh[half][:, :bs * 64],
                                in1=b2sb[:, :bs * 64], op=Alu.add)
                            mul_in = eb
                        msb = mpool.tile([128, BS * 64], bf)
                        msbh.append(msb)
                        nwoff = (half * T + t0) * 64
                        nc.vector.tensor_tensor(
                            out=msb[:, :bs * 64], in0=mul_in[:, :bs * 64],
                            in1=nwpair[:, nwoff:nwoff + bs * 64], op=Alu.mult)
                        if level < 5:
                            continue
                        oh = ohpool.tile([128, BS, 128], bf)
                        ohh.append(oh)
                        if oh_engine == "ts":
                            for j in range(bs):
                                t = t0 + j
                                nc.vector.tensor_scalar(
                                    out=oh[:, j], in0=iota[:],
                                    scalar1=dwsb[:, wc + t:wc + t + 1],
                                    scalar2=None, op0=Alu.is_equal)
                        else:
                            nc.vector.tensor_tensor(
                                out=oh[:, :bs],
                                in0=dwsb[:, wc + t0:wc + t1, None].to_broadcast([128, bs, 128]),
                                in1=iota[:, None, :].to_broadcast([128, bs, 128]),
                                op=Alu.is_equal)
                    if level < 6:
                        continue
                    for j in range(bs):
                        t = t0 + j
                        for half in (0, 1):
                            r0 = 64 * half
                            nc.tensor.matmul(
                                out=hps[r0:r0 + 64, :],
                                lhsT=msbh[half][:, j * 64:(j + 1) * 64],
                                rhs=ohh[half][:, j],
                                start=(t == 0), stop=(t == T - 1),
                                tile_position=(0, r0),
                            )
                if level >= 6:
                    hsb = hsbpool.tile([128, 128], f32)
                    nc.vector.tensor_copy(out=hsb[:], in_=hps[:])
                    nc.sync.dma_start(out=hout[:, p * 128:(p + 1) * 128], in_=hsb[:])

            def emit_all():
                for p in range(NPAIR):
                    emit_pair(p)

            if repeats > 1:
                with tc.For_i(0, repeats, 1):
                    emit_all()
            else:
                emit_all()
    nc.compile()
    return nc


def _get_nc(T, level=6, repeats=1):
    key = (T, level, repeats)
    if key not in _NC_CACHE:
        _NC_CACHE[key] = _build(
            T, level=level, repeats=repeats,
            oh_engine=os.environ.get("OH_ENGINE", "dve"),
            badd=os.environ.get("BADD", "dve"))
    return _NC_CACHE[key]


def _prepare(node_weight, rbf_out, W1, b1, W2, b2, src, dst):
    """Host-side shard/sort/pack. Returns (T, in_maps)."""
    order = np.argsort(dst, kind="stable")
    d_s = dst[order]
    s_s = src[order].astype(np.int32)
    rbf_s = rbf_out[order].astype(bf16)

    core = d_s // NPC
    local = d_s - core * NPC
    win = local >> 7
    rel = (local & 127).astype(np.float32)
    gid = core * NWIN + win
    counts = np.bincount(gid, minlength=CORES * NWIN)
    starts = np.zeros(CORES * NWIN + 1, dtype=np.int64)
    np.cumsum(counts, out=starts[1:])
    T = max(2, int(np.ceil(counts.max() / 128.0)))

    S = NPAIR * T * 128
    WT = NWIN * T
    GC = NPAIR * 2 * T * 64

    w1bf = np.concatenate([W1, W1], axis=0).astype(bf16)          # [128, 64]
    w2bf = np.concatenate([2.0 * W2, 2.0 * W2], axis=0).astype(bf16)
    b1h = np.tile((0.5 * b1)[:, None], (2, 1)).astype(np.float32)  # [128, 1]
    b2r = np.tile(b2, (128, BS)).astype(np.float32)                # [128, BS*64]
    nw_bf = node_weight.astype(bf16)

    in_maps = []
    for c in range(CORES):
        xTc = np.zeros((128, S), dtype=bf16)
        nwc = np.zeros((128, GC), dtype=bf16)
        dwc = np.full((128, WT), -1.0, dtype=np.float32)
        for w in range(NWIN):
            g = c * NWIN + w
            e0, e1 = starts[g], starts[g + 1]
            cnt = e1 - e0
            if cnt == 0:
                continue
            pr = w // 2
            half = w % 2
            rows = slice(0, 64) if half == 0 else slice(64, 128)
            cbase = pr * T * 128
            # feature-major features for this window's edges
            xTc[rows, cbase:cbase + cnt] = rbf_s[e0:e1].T
            # pre-gathered node weights, edge-major slot layout:
            # pair block, then half, tile t at cols (t*64..); row r = slot%128
            gw = np.zeros((T * 128, DIM), dtype=bf16)
            gw[:cnt] = nw_bf[s_s[e0:e1]]
            gbase = (pr * 2 * T + half * T) * 64
            nwc[:, gbase:gbase + T * 64] = (
                gw.reshape(T, 128, DIM).transpose(1, 0, 2).reshape(128, T * 64))
            rl = rel[e0:e1]
            ntile = (cnt + 127) // 128
            for t in range(ntile):
                a, b_ = t * 128, min((t + 1) * 128, cnt)
                dwc[0:b_ - a, w * T + t] = rl[a:b_]
        in_maps.append({
            "xT": xTc, "nwp": nwc, "dstw": dwc.astype(bf16),
            "W1r": w1bf, "W2r": w2bf, "b1h": b1h, "b2r": b2r,
        })
    return T, in_maps


def kernel(node_weight, rbf_out, W1, b1, W2, b2, src, dst, n_nodes=N_NODES):
    global LAST_RESULTS
    node_weight = np.asarray(node_weight, dtype=np.float32)
    rbf_out = np.asarray(rbf_out, dtype=np.float32)
    W1 = np.asarray(W1, dtype=np.float32)
    b1 = np.asarray(b1, dtype=np.float32)
    W2 = np.asarray(W2, dtype=np.float32)
    b2 = np.asarray(b2, dtype=np.float32)
    src = np.asarray(src).astype(np.int64)
    dst = np.asarray(dst).astype(np.int64)
    assert int(n_nodes) == N_NODES and src.shape[0] == N_EDGES

    T, in_maps = _prepare(node_weight, rbf_out, W1, b1, W2, b2, src, dst)
    nc = _get_nc(T)

    from concourse.bass_utils import run_bass_kernel_spmd
    trace = bool(int(os.environ.get("KTRACE", "0")))
    res = run_bass_kernel_spmd(nc, in_maps, list(range(CORES)), trace=trace)
    LAST_RESULTS = res

    h = np.empty((CORES, NPC, DIM), dtype=np.float32)
    for c in range(CORES):
        ho = res.results[c]["h_out"]          # [128, NPAIR*128]
        hp = ho.reshape(128, NPAIR, 128)
        # window 2p -> rows 0:64, window 2p+1 -> rows 64:128
        full = np.empty((NWIN * WIN, DIM), dtype=np.float32)
        for half in (0, 1):
            blk = hp[64 * half:64 * half + 64]            # [64, NPAIR, 128]
            blk = blk.transpose(1, 2, 0)                  # [NPAIR, 128n, 64f]
            w_idx = np.arange(half, NWIN, 2)
            node0 = w_idx * WIN
            for k, n0 in enumerate(node0):
                full[n0:n0 + WIN] = blk[k]
        h[c] = full[:NPC]
    return h.reshape(N_NODES, DIM)

